# revision 23
# baseline (speedup 1.0000x reference)
"""Trainium2 Bass kernel for nn_MultiModalFusionModelWithAblation.

Strategy: pure data-parallel over 8 NeuronCores (B=16384 -> 2048 rows/core).
Row-major home layout; activations transposed via xbar DMA where a matmul
needs them feature-major (stationary lhsT), bf16 matmul inputs, fp32 PSUM.

v2 structure (vs the v1 baseline):
  - all weights pre-cast to bf16 on HOST (ml_dtypes) and DMAed directly --
    no on-chip cast preamble.
  - super-tiles of SUPER=2 row-tiles: all DVE/Scalar stages run batched
    across the super-tile so activation-table reloads amortize and fixed
    per-instruction overheads shrink.
  - projection relu runs on DVE as tensor_scalar(max) with accum_out
    giving the LN1 row-sum for free (Scalar no longer does the big
    PSUM->SBUF relus).
  - LN2 stats via STT accum_out (sum / sum-of-squares) instead of
    bn_stats; single batched Sqrt per super-tile.
  - sigmoid via Exp + DVE reciprocal (no Sigmoid table).
  - aux logits/scores DMAed straight into transposed layout (strided
    SWDGE) -- no pad/copy/xbar-transpose.
  - pooled attention: per-modality TT chains on contiguous xv slices;
    query 0 on DVE, query 1 on GpSimd (parallel engines).

Host-side algebra (exact, weight-space only) -- unchanged from v1:
  - gat_W folded into the MHA score/value projections (GS / GV).
  - LN affines folded where linear; LN1 rstd dropped entirely (LN2 is
    invariant to per-row positive scales when adapter biases are zero).
  - biases via K=1 ones-outer-product matmuls, skipped when zero.
"""
import sys
import os

sys.path.insert(0, "/opt/trn_rl_repo")

import numpy as np
import orjson
import ml_dtypes
from contextlib import ExitStack

import concourse.bass as bass
import concourse.tile as tile
from concourse import mybir

# ----------------------------------------------------------------------------
# walrus on this toolchain rejects >1 sync-wait per instruction; split excess
# waits onto NoOp carriers on the same engine queue (in-order => equivalent).
_FIXN = [0]


def _fix_bir_waits(d):
    for f in d.get("functions", []):
        for b in f.get("blocks", []):
            insts = b.get("instructions", [])
            if not any(
                len(((i.get("sync_info") or {}).get("on_wait") or [])) > 1
                for i in insts
            ):
                continue
            new = []
            for inst in insts:
                si = inst.get("sync_info")
                waits = (si or {}).get("on_wait") or []
                if len(waits) > 1:
                    for w in waits[:-1]:
                        _FIXN[0] += 1
                        new.append({
                            "engine": inst["engine"], "ins": [], "outs": [],
                            "name": f"wfix-{_FIXN[0]}", "opcode": "NoOp",
                            "debug": inst.get("debug", 0),
                            "sync_info": {"on_update": [], "on_wait": [w]},
                        })
                    si["on_wait"] = [waits[-1]]
                new.append(inst)
            b["instructions"] = new
    return d


if not getattr(bass.Bass, "_waitfix_installed", False):
    _orig_tjb = bass.Bass.to_json_bytes

    def _patched_tjb(self):
        return orjson.dumps(_fix_bir_waits(orjson.loads(_orig_tjb(self))))

    bass.Bass.to_json_bytes = _patched_tjb
    bass.Bass._waitfix_installed = True

# ----------------------------------------------------------------------------
H = 512
NH = 8
HD = 64
NMOD = 5
IN_DIMS = [2048, 1024, 1536, 512, 512]
MODS = ["body", "face", "scene", "audio", "text"]
B_FULL = 16384
NCORES = 8
B_CORE = B_FULL // NCORES          # 2048
NT = B_CORE // 128                 # 16 row tiles per core
SUPER = 1                          # row tiles per super-tile
NSB = NT // SUPER                  # super-tiles per core
ALPHA = 0.2
EPS = 1e-5

F32 = mybir.dt.float32
BF16 = mybir.dt.bfloat16
AF = mybir.ActivationFunctionType
AL = mybir.AluOpType


def _build_nc(flags):
    nc = bass.Bass("TRN2", target_bir_lowering=False, debug=False,
                   num_devices=NCORES)

    # ---- dram io (all weights pre-cast to bf16 on host) ----
    feat_d = [nc.dram_tensor(f"feat_{m}", [B_CORE, ind], F32, kind="ExternalInput")
              for m, ind in zip(MODS, IN_DIMS)]
    wp_d = [nc.dram_tensor(f"wp_{m}", [ind, H], BF16, kind="ExternalInput")
            for m, ind in zip(MODS, IN_DIMS)]
    aw1_d = nc.dram_tensor("aw1", [NMOD, H, H // 2], BF16, kind="ExternalInput")
    aw2_d = nc.dram_tensor("aw2", [NMOD, H // 2, H], BF16, kind="ExternalInput")
    gv_d = nc.dram_tensor("gv", [H, H], BF16, kind="ExternalInput")
    gs_d = nc.dram_tensor("gs", [H, 18], BF16, kind="ExternalInput")
    wo_d = nc.dram_tensor("wo", [H, H], BF16, kind="ExternalInput")
    pc_d = nc.dram_tensor("pc", [H, 24], BF16, kind="ExternalInput")
    elp5_d = nc.dram_tensor("elp5", [35, H], BF16, kind="ExternalInput")
    plp5_d = nc.dram_tensor("plp5", [25, H], BF16, kind="ExternalInput")
    logits_d = nc.dram_tensor("logits", [NMOD, B_CORE, 7], F32, kind="ExternalInput")
    scores5_d = nc.dram_tensor("scores5", [NMOD, B_CORE, 5], F32, kind="ExternalInput")
    # optional bias rows (always declared; tiny)
    bp_d = nc.dram_tensor("bp", [NMOD, H], BF16, kind="ExternalInput")
    ab2_d = nc.dram_tensor("ab2e", [NMOD, H], BF16, kind="ExternalInput")
    rc_d = nc.dram_tensor("rc", [2, H], BF16, kind="ExternalInput")
    pcb_d = nc.dram_tensor("pcb", [1, 24], BF16, kind="ExternalInput")
    ck_d = nc.dram_tensor("ck", [1, 16], BF16, kind="ExternalInput")
    out_d = nc.dram_tensor("out", [B_CORE, 12], F32, kind="ExternalOutput")

    NK = [ind // 128 for ind in IN_DIMS]
    T = SUPER

    with tile.TileContext(nc) as tc, ExitStack() as ctx:
        wpool = ctx.enter_context(tc.tile_pool(name="weights", bufs=1))
        sb = ctx.enter_context(tc.tile_pool(name="work", bufs=1))
        ps = ctx.enter_context(tc.tile_pool(name="psum", bufs=1, space="PSUM"))

        # ---- one-time weight loads: direct bf16 HWDGE on the scalar queue
        def _wload(dram_ap, shape, tag):
            t = wpool.tile(shape, BF16, tag=tag)
            nc.scalar.dma_start(t[:], dram_ap)
            return t

        wp_bf = [
            _wload(wp_d[m].ap().rearrange("(k c) n -> c k n", c=128),
                   [128, NK[m], H], f"wp{m}")
            for m in range(NMOD)
        ]
        aw1_bf = _wload(aw1_d.ap().rearrange("m (k c) n -> c (m k) n", c=128),
                        [128, NMOD * 4, H // 2], "aw1")
        aw1_bf = aw1_bf[:].rearrange("c (m k) n -> c m k n", m=NMOD)
        aw2_bf = _wload(aw2_d.ap().rearrange("m (k c) n -> c (m k) n", c=128),
                        [128, NMOD * 2, H], "aw2")
        aw2_bf = aw2_bf[:].rearrange("c (m k) n -> c m k n", m=NMOD)
        gv_bf = _wload(gv_d.ap().rearrange("(k c) n -> c k n", c=128),
                       [128, 4, H], "gv")
        gs_bf = _wload(gs_d.ap().rearrange("(k c) n -> c k n", c=128),
                       [128, 4, 18], "gs")
        wo_bf = _wload(wo_d.ap().rearrange("(k c) n -> c k n", c=128),
                       [128, 4, H], "wo")
        pc_bf = _wload(pc_d.ap().rearrange("(k c) n -> c k n", c=128),
                       [128, 4, 24], "pc")
        elp5_bf = _wload(elp5_d.ap(), [35, H], "elp5")
        plp5_bf = _wload(plp5_d.ap(), [25, H], "plp5")

        eps_t = wpool.tile([128, 1], F32, tag="eps")
        nc.vector.memset(eps_t[:], EPS)

        ones1 = None
        if any([flags["bp"], flags["ab2"], flags["rc"], flags["pcb"],
                flags["ck"]]):
            ones1 = wpool.tile([1, 128], BF16, tag="ones1")
            nc.vector.memset(ones1[:], 1.0)

        def _bias_row(dram_ap, n, tag):
            t = wpool.tile([1, n], BF16, tag=tag)
            nc.gpsimd.dma_start(t[:], dram_ap)
            return t

        bp_bf = _bias_row(bp_d.ap().rearrange("m n -> 1 (m n)"), NMOD * H, "bp") \
            if flags["bp"] else None
        ab2_bf = _bias_row(ab2_d.ap().rearrange("m n -> 1 (m n)"), NMOD * H, "ab2") \
            if flags["ab2"] else None
        rc_bf = _bias_row(rc_d.ap().rearrange("q n -> 1 (q n)"), 2 * H, "rc") \
            if flags["rc"] else None
        pcb_bf = _bias_row(pcb_d.ap()[:], 24, "pcb") if flags["pcb"] else None
        ck_t = None
        if flags["ck"]:
            ck_row = _bias_row(ck_d.ap()[:], 16, "ckrow")
            ck_ps = ps.tile([128, H], F32, tag="psB")
            nc.tensor.matmul(ck_ps[:, 0:16], lhsT=ones1[:], rhs=ck_row[:],
                             start=True, stop=True)
            ck_t = wpool.tile([128, 16], F32, tag="ckt")
            nc.vector.tensor_copy(out=ck_t[:], in_=ck_ps[:, 0:16])

        # persistent zero-padded staging for the aux-logit transposes
        auxpad = []
        for i in range(2):
            t = wpool.tile([128, T, 2, 128], BF16, tag=f"auxpad{i}")
            nc.vector.memset(t[:], 0.0)
            auxpad.append(t)

        # xbar queue alternation for DMA transposes
        _tq = [0]

        def _tqueue():
            _tq[0] ^= 1
            return nc.sync

        # ---------------- pipeline stages (per super-tile of T row-tiles) ---
        def emit_A(sb_i):
            """aux + feat loads, transposes, projection, relu+rowsum."""
            st = {"sb": sb_i}
            tiles = [sb_i * T + t for t in range(T)]
            st["r0"] = [ti * 128 for ti in tiles]

            h4 = sb.tile([128, T, NMOD, H], BF16, tag="h4", bufs=2)
            hsum = sb.tile([128, T * NMOD], F32, tag="hsum", bufs=2)
            for t, r0 in enumerate(st["r0"]):
                for grp in ([0], [2], [1, 3, 4]):
                    gw = sum(IN_DIMS[m] for m in grp)
                    fz = sb.tile([128, IN_DIMS[0]], BF16, tag="fz", bufs=2)
                    off = 0
                    for m in grp:
                        nc.gpsimd.dma_start(
                            fz[:, off:off + IN_DIMS[m]],
                            feat_d[m].ap()[r0:r0 + 128, :])
                        off += IN_DIMS[m]
                    fT = sb.tile([128, NK[0], 128], BF16, tag="fT", bufs=2)
                    _tqueue().dma_start(fT[:, :gw // 128, :], fz[:, :gw],
                                        transpose=True)
                    koff = 0
                    for m in grp:
                        nk = NK[m]
                        h_ps = ps.tile([128, H], F32, tag="psA", bufs=2)
                        if flags["bp"]:
                            nc.tensor.matmul(h_ps[:], lhsT=ones1[:],
                                             rhs=bp_bf[:, m * H:(m + 1) * H],
                                             start=True, stop=False)
                        for k in range(nk):
                            nc.tensor.matmul(h_ps[:], lhsT=fT[:, koff + k, :],
                                             rhs=wp_bf[m][:, k, :],
                                             start=(k == 0 and not flags["bp"]),
                                             stop=(k == nk - 1))
                        koff += nk
                        # relu + row-sum in one DVE op
                        idx = t * NMOD + m
                        nc.vector.tensor_scalar(
                            out=h4[:, t, m, :], in0=h_ps[:], scalar1=0.0,
                            scalar2=0.0, op0=AL.max, op1=AL.add,
                            accum_out=hsum[:, idx:idx + 1])
            negmu = sb.tile([128, T * NMOD], F32, tag="negmu", bufs=2)
            nc.vector.tensor_scalar_mul(negmu[:], hsum[:], -1.0 / H)
            st["h4"] = h4
            st["negmu"] = negmu
            return st

        def emit_LN1(st):
            h4, negmu = st["h4"], st["negmu"]
            hln = sb.tile([128, T, NMOD, H], BF16, tag="hln", bufs=2)
            for t in range(T):
                for m in range(NMOD):
                    idx = t * NMOD + m
                    nc.vector.tensor_scalar(
                        out=hln[:, t, m, :], in0=h4[:, t, m, :],
                        scalar1=negmu[:, idx:idx + 1], scalar2=None,
                        op0=AL.add)
            hT = sb.tile([128, T, NMOD * 4, 128], BF16, tag="hT", bufs=2)
            for t in range(T):
                _tqueue().dma_start(
                    hT[:, t], hln[:, t].rearrange("p m h -> p (m h)"),
                    transpose=True)
            st["hln"] = hln
            st["hT"] = hT
            return st

        def emit_C(st):
            hln, hT = st["hln"], st["hT"]
            assert not flags.get("ab1", False)
            # adapter hidden, feature-major, batched over the super-tile rows
            zT = sb.tile([128, NMOD * 2, T * 128], BF16, tag="zT", bufs=2)
            for m in range(NMOD):
                for c in range(2):
                    z_ps = ps.tile([128, H], F32, tag="psB", bufs=2)
                    for k in range(4):
                        nc.tensor.matmul(
                            z_ps[:, :T * 128],
                            lhsT=aw1_bf[:, m, k, c * 128:(c + 1) * 128],
                            rhs=hT[:, :, m * 4 + k, :],
                            start=(k == 0), stop=(k == 3))
                    nc.scalar.activation(zT[:, m * 2 + c, :],
                                         z_ps[:, :T * 128], AF.Relu)
            # adapter out + residual; LN2 stats via accum_out
            u4 = sb.tile([128, T, NMOD, H], BF16, tag="u4", bufs=2)
            su = sb.tile([128, T * NMOD], F32, tag="su", bufs=2)
            su2 = sb.tile([128, T * NMOD], F32, tag="su2", bufs=2)
            for t in range(T):
                for m in range(NMOD):
                    a2_ps = ps.tile([128, H], F32, tag="psC", bufs=2)
                    if flags["ab2"]:
                        nc.tensor.matmul(a2_ps[:], lhsT=ones1[:],
                                         rhs=ab2_bf[:, m * H:(m + 1) * H],
                                         start=True, stop=False)
                    for k in range(2):
                        nc.tensor.matmul(
                            a2_ps[:],
                            lhsT=zT[:, m * 2 + k, t * 128:(t + 1) * 128],
                            rhs=aw2_bf[:, m, k, :],
                            start=(k == 0 and not flags["ab2"]),
                            stop=(k == 1))
                    idx = t * NMOD + m
                    nc.vector.scalar_tensor_tensor(
                        out=u4[:, t, m, :], in0=a2_ps[:], scalar=1.0,
                        in1=hln[:, t, m, :], op0=AL.mult, op1=AL.add,
                        accum_out=su[:, idx:idx + 1])
                    s2 = sb.tile([128, H], BF16, tag="scr", bufs=1)
                    nc.vector.scalar_tensor_tensor(
                        out=s2[:], in0=u4[:, t, m, :], scalar=1.0,
                        in1=u4[:, t, m, :], op0=AL.mult, op1=AL.mult,
                        accum_out=su2[:, idx:idx + 1])
            # mu, var, rstd (batched tiny ops + one Sqrt)
            mu = sb.tile([128, T * NMOD], F32, tag="mu", bufs=2)
            nc.vector.tensor_scalar_mul(mu[:], su[:], 1.0 / H)
            m2 = sb.tile([128, T * NMOD], F32, tag="m2", bufs=2)
            nc.vector.tensor_tensor(out=m2[:], in0=mu[:], in1=mu[:],
                                    op=AL.mult)
            var = sb.tile([128, T * NMOD], F32, tag="var", bufs=2)
            nc.vector.scalar_tensor_tensor(
                out=var[:], in0=su2[:], scalar=1.0 / H, in1=m2[:],
                op0=AL.mult, op1=AL.subtract)
            sd = sb.tile([128, T * NMOD], F32, tag="sd", bufs=2)
            nc.scalar.activation(sd[:], var[:], AF.Sqrt, bias=eps_t[:])
            rstd = sb.tile([128, T * NMOD], F32, tag="rstd", bufs=2)
            nc.vector.reciprocal(rstd[:], sd[:])
            st["u4"] = u4
            st["mu"] = mu
            st["rstd"] = rstd
            return st

        def emit_LN2(st):
            u4, mu, rstd = st["u4"], st["mu"], st["rstd"]
            xT = sb.tile([128, T, NMOD * 4, 128], BF16, tag="xT", bufs=2)
            for t in range(T):
                xm = sb.tile([128, NMOD, H], BF16, tag="xm", bufs=1)
                for m in range(NMOD):
                    idx = t * NMOD + m
                    nc.vector.tensor_scalar(
                        out=xm[:, m, :], in0=u4[:, t, m, :],
                        scalar1=mu[:, idx:idx + 1],
                        scalar2=rstd[:, idx:idx + 1],
                        op0=AL.subtract, op1=AL.mult)
                _tqueue().dma_start(
                    xT[:, t], xm[:].rearrange("p m h -> p (m h)"),
                    transpose=True)
            st["xT"] = xT
            return st

        def emit_E(st):
            xT = st["xT"]
            xv4 = sb.tile([128, T, NMOD, H], BF16, tag="xv4", bufs=2)
            xss = sb.tile([128, T, NMOD, 18], F32, tag="xss", bufs=2)
            for t in range(T):
                for m in range(NMOD):
                    xv_ps = ps.tile([128, H], F32, tag="psD", bufs=2)
                    xs_ps = ps.tile([128, H], F32, tag="psB", bufs=2)
                    for k in range(4):
                        nc.tensor.matmul(xv_ps[:], lhsT=xT[:, t, m * 4 + k, :],
                                         rhs=gv_bf[:, k, :],
                                         start=(k == 0), stop=(k == 3))
                        nc.tensor.matmul(xs_ps[:, 0:18],
                                         lhsT=xT[:, t, m * 4 + k, :],
                                         rhs=gs_bf[:, k, :],
                                         start=(k == 0), stop=(k == 3))
                    nc.scalar.activation(xv4[:, t, m, :], xv_ps[:], AF.Copy)
                    nc.vector.tensor_copy(out=xss[:, t, m, :],
                                          in_=xs_ps[:, 0:18])
            st["xv4"] = xv4
            st["xss"] = xss
            return st

        def emit_attn(st):
            xss = st["xss"]
            s1 = xss[:, :, :, 16]                       # [128,T,5]
            s2 = xss[:, :, :, 17]
            e4 = sb.tile([128, T, 5, 5], F32, tag="e4", bufs=1)
            nc.vector.tensor_tensor(
                out=e4[:],
                in0=s2[:, :, None, :].broadcast_to([128, T, 5, 5]),
                in1=s1[:, :, :, None].broadcast_to([128, T, 5, 5]),
                op=AL.add)
            el = sb.tile([128, T, 25], F32, tag="el", bufs=1)
            nc.vector.scalar_tensor_tensor(
                out=el[:], in0=e4[:].rearrange("p t a b -> p t (a b)"),
                scalar=ALPHA,
                in1=e4[:].rearrange("p t a b -> p t (a b)"),
                op0=AL.mult, op1=AL.max)
            ex = sb.tile([128, T, 5, 5], F32, tag="ex", bufs=1)
            nc.scalar.activation(ex[:].rearrange("p t a b -> p t (a b)"),
                                 el[:], AF.Exp)
            den = sb.tile([128, T, 5], F32, tag="den", bufs=1)
            nc.vector.tensor_reduce(out=den[:], in_=ex[:],
                                    axis=mybir.AxisListType.X, op=AL.add)
            rden = sb.tile([128, T, 5], F32, tag="rden", bufs=1)
            nc.vector.reciprocal(rden[:], den[:])
            attn = sb.tile([128, T, 5, 5], F32, tag="attn", bufs=1)
            nc.vector.tensor_tensor(
                out=attn[:], in0=ex[:],
                in1=rden[:, :, :, None].broadcast_to([128, T, 5, 5]),
                op=AL.mult)

            tmp400 = sb.tile([128, T, 16, 5, 5], BF16, tag="tmp400", bufs=1)
            S4 = sb.tile([128, T, 16, 5], F32, tag="S4", bufs=1)
            for t in range(T):
                nc.vector.tensor_tensor(
                    out=tmp400[:, t],
                    in0=xss[:, t, :, 0:16].rearrange("p j q -> p q j")
                        [:, :, None, :].broadcast_to([128, 16, 5, 5]),
                    in1=attn[:, t][:, None, :, :].broadcast_to([128, 16, 5, 5]),
                    op=AL.mult)
                nc.vector.tensor_reduce(out=S4[:, t], in_=tmp400[:, t],
                                        axis=mybir.AxisListType.X, op=AL.add)
            if flags["ck"]:
                nc.vector.tensor_tensor(
                    out=S4[:], in0=S4[:],
                    in1=ck_t[:][:, None, :, None]
                        .broadcast_to([128, T, 16, 5]), op=AL.add)
            ES = sb.tile([128, T, 16, 5], F32, tag="ES", bufs=1)
            nc.scalar.activation(ES[:].rearrange("p t a b -> p t (a b)"),
                                 S4[:].rearrange("p t a b -> p t (a b)"),
                                 AF.Exp)
            den16 = sb.tile([128, T, 16], F32, tag="den16", bufs=1)
            nc.vector.tensor_reduce(out=den16[:], in_=ES[:],
                                    axis=mybir.AxisListType.X, op=AL.add)
            rden16 = sb.tile([128, T, 16], F32, tag="rden16", bufs=1)
            nc.vector.reciprocal(rden16[:], den16[:])
            P4 = sb.tile([128, T, 16, 5], BF16, tag="P4", bufs=1)
            nc.vector.tensor_tensor(
                out=P4[:], in0=ES[:],
                in1=rden16[:, :, :, None].broadcast_to([128, T, 16, 5]),
                op=AL.mult)
            tmp2 = sb.tile([128, T, 16, 5, 5], BF16, tag="tmp400", bufs=1)
            W4 = sb.tile([128, T, 16, 5], BF16, tag="W4", bufs=2)
            for t in range(T):
                nc.vector.tensor_tensor(
                    out=tmp2[:, t],
                    in0=P4[:, t][:, :, None, :].broadcast_to([128, 16, 5, 5]),
                    in1=attn[:, t].rearrange("p n j -> p j n")
                        [:, None, :, :].broadcast_to([128, 16, 5, 5]),
                    op=AL.mult)
                with nc.allow_low_precision("5-term pooled-attn sums"):
                    nc.vector.tensor_reduce(out=W4[:, t], in_=tmp2[:, t],
                                            axis=mybir.AxisListType.X, op=AL.add)
            st["W4"] = W4
            return st

        def _pool_q(eng, xv4, W4, o4, q, tags):
            """o4[:, :, q, :] = sum_j W4[:, :, q-heads, j] * xv4[:, :, j, :]"""
            def wv(j):
                return W4[:, :, q * 8:(q + 1) * 8, j:j + 1] \
                    .broadcast_to([128, T, 8, HD])

            def xv(j):
                return xv4[:, :, j, :].rearrange("p t (h d) -> p t h d", h=8)

            pa = sb.tile([128, T, 8, HD], BF16, tag=tags[0], bufs=1)
            pb = sb.tile([128, T, 8, HD], BF16, tag=tags[1], bufs=1)
            with nc.allow_low_precision("5-term pooled-attn sums"):
                eng.tensor_tensor(out=pa[:], in0=xv(0), in1=wv(0), op=AL.mult)
                eng.tensor_tensor(out=pb[:], in0=xv(1), in1=wv(1), op=AL.mult)
                eng.tensor_tensor(out=pa[:], in0=pa[:], in1=pb[:], op=AL.add)
                pb2 = sb.tile([128, T, 8, HD], BF16, tag=tags[1], bufs=1)
                eng.tensor_tensor(out=pb2[:], in0=xv(2), in1=wv(2), op=AL.mult)
                eng.tensor_tensor(out=pa[:], in0=pa[:], in1=pb2[:], op=AL.add)
                pb3 = sb.tile([128, T, 8, HD], BF16, tag=tags[1], bufs=1)
                eng.tensor_tensor(out=pb3[:], in0=xv(3), in1=wv(3), op=AL.mult)
                eng.tensor_tensor(out=pa[:], in0=pa[:], in1=pb3[:], op=AL.add)
                pb4 = sb.tile([128, T, 8, HD], BF16, tag=tags[1], bufs=1)
                eng.tensor_tensor(out=pb4[:], in0=xv(4), in1=wv(4), op=AL.mult)
                eng.tensor_tensor(
                    out=o4[:, :, q, :].rearrange("p t (h d) -> p t h d", h=8),
                    in0=pa[:], in1=pb4[:], op=AL.add)

        def emit_pool(st):
            xv4, W4 = st["xv4"], st["W4"]
            # aux logits/scores: contiguous row-major loads into the
            # zero-padded staging, one xbar transpose per super-tile
            pad = auxpad[st["sb"] % 2]
            for t, r0 in enumerate(st["r0"]):
                lg = sb.tile([128, NMOD, 7], F32, tag="lg", bufs=2)
                nc.gpsimd.dma_start(
                    lg[:], logits_d.ap()[:, r0:r0 + 128, :]
                    .rearrange("m r c -> r m c"))
                nc.vector.tensor_copy(out=pad[:, t, 0, 0:35],
                                      in_=lg[:].rearrange("p m c -> p (m c)"))
                sc = sb.tile([128, NMOD, 5], F32, tag="sc", bufs=2)
                nc.gpsimd.dma_start(
                    sc[:], scores5_d.ap()[:, r0:r0 + 128, :]
                    .rearrange("m r c -> r m c"))
                nc.vector.tensor_copy(out=pad[:, t, 1, 0:25],
                                      in_=sc[:].rearrange("p m c -> p (m c)"))
            auxT = sb.tile([128, T, 2, 128], BF16, tag="auxT", bufs=3)
            nc.sync.dma_start(auxT[:].rearrange("p t a b -> p (t a) b"),
                              pad[:].rearrange("p t a b -> p (t a b)"),
                              transpose=True)
            st["auxT"] = auxT
            o4 = sb.tile([128, T, 2, H], BF16, tag="o4", bufs=2)
            _pool_q(nc.vector, xv4, W4, o4, 0, ("vpa", "vpb"))
            _pool_q(nc.vector, xv4, W4, o4, 1, ("vpa", "vpb"))
            st["o4"] = o4
            return st

        def emit_out(st):
            o4, auxT = st["o4"], st["auxT"]
            oT = sb.tile([128, T, 8, 128], BF16, tag="oT", bufs=2)
            for t in range(T):
                _tqueue().dma_start(
                    oT[:, t], o4[:, t].rearrange("p a b -> p (a b)"),
                    transpose=True)
            rep4 = sb.tile([128, T, 2, H], BF16, tag="rep4", bufs=2)
            n2 = sb.tile([128, T * 2], F32, tag="n2", bufs=2)
            for t in range(T):
                for q in range(2):
                    repr_ps = ps.tile([128, H], F32, tag="psD", bufs=2)
                    if flags["rc"]:
                        nc.tensor.matmul(repr_ps[:], lhsT=ones1[:],
                                         rhs=rc_bf[:, q * H:(q + 1) * H],
                                         start=True, stop=False)
                    for k in range(4):
                        nc.tensor.matmul(repr_ps[:], lhsT=oT[:, t, q * 4 + k, :],
                                         rhs=wo_bf[:, k, :],
                                         start=(k == 0 and not flags["rc"]),
                                         stop=False)
                    if q == 0:
                        nc.tensor.matmul(repr_ps[:],
                                         lhsT=auxT[0:35, t, 0, :],
                                         rhs=elp5_bf[:], start=False, stop=True)
                    else:
                        nc.tensor.matmul(repr_ps[:],
                                         lhsT=auxT[0:25, t, 1, :],
                                         rhs=plp5_bf[:], start=False, stop=True)
                    nc.scalar.activation(rep4[:, t, q, :], repr_ps[:], AF.Copy)
                    sq = sb.tile([128, H], BF16, tag="scr2", bufs=1)
                    idx = t * 2 + q
                    nc.vector.scalar_tensor_tensor(
                        out=sq[:], in0=rep4[:, t, q, :], scalar=1.0,
                        in1=rep4[:, t, q, :], op0=AL.mult, op1=AL.mult,
                        accum_out=n2[:, idx:idx + 1])
            nrm = sb.tile([128, T * 2], F32, tag="nrm", bufs=2)
            nc.scalar.activation(nrm[:], n2[:], AF.Sqrt)
            nc.vector.tensor_scalar_max(nrm[:], nrm[:], 1e-8)
            rn = sb.tile([128, T * 2], F32, tag="rn", bufs=2)
            nc.vector.reciprocal(rn[:], nrm[:])
            negrn = sb.tile([128, T * 2], F32, tag="negrn", bufs=2)
            nc.vector.tensor_scalar_mul(negrn[:], rn[:], -1.0)

            rT = sb.tile([128, T, 8, 128], BF16, tag="rT", bufs=2)
            for t in range(T):
                _tqueue().dma_start(
                    rT[:, t], rep4[:, t].rearrange("p a b -> p (a b)"),
                    transpose=True)
            pred4 = sb.tile([128, T, 24], F32, tag="pred4", bufs=2)
            for t in range(T):
                pred_ps = ps.tile([128, H], F32, tag="psB", bufs=2)
                if flags["pcb"]:
                    nc.tensor.matmul(pred_ps[:, 0:24], lhsT=ones1[:],
                                     rhs=pcb_bf[:], start=True, stop=False)
                for q in range(2):
                    cols = slice(0, 14) if q == 0 else slice(14, 24)
                    for k in range(4):
                        nc.tensor.matmul(pred_ps[:, cols],
                                         lhsT=rT[:, t, q * 4 + k, :],
                                         rhs=pc_bf[:, k, cols],
                                         start=(k == 0 and not flags["pcb"]),
                                         stop=(k == 3))
                nc.vector.tensor_copy(out=pred4[:, t, :],
                                      in_=pred_ps[:, 0:24])

            outt = sb.tile([128, T, 12], F32, tag="outt", bufs=2)
            # emo half: pred[0:7]*0.5-folded + cos*0.5-folded
            c7 = sb.tile([128, T, 7], F32, tag="c7", bufs=2)
            nc.vector.tensor_tensor(
                out=c7[:], in0=pred4[:, :, 7:14],
                in1=rn[:].rearrange("p (t q) -> p t q", t=T)[:, :, 0:1]
                    .broadcast_to([128, T, 7]),
                op=AL.mult)
            nc.vector.tensor_tensor(out=outt[:, :, 0:7], in0=c7[:],
                                    in1=pred4[:, :, 0:7], op=AL.add)
            # pkl half: (sigmoid(pred14:19) + sigmoid(cos))*0.5 via Exp
            Ec = sb.tile([128, T, 5], F32, tag="Ec", bufs=2)
            for t in range(T):
                idx = t * 2 + 1
                nc.scalar.activation(Ec[:, t, :], pred4[:, t, 19:24], AF.Exp,
                                     scale=negrn[:, idx:idx + 1])
            Ep = sb.tile([128, T, 5], F32, tag="Ep", bufs=2)
            nc.scalar.activation(Ep[:], pred4[:, :, 14:19], AF.Exp, scale=-1.0)
            dc = sb.tile([128, T, 5], F32, tag="dc", bufs=2)
            nc.vector.tensor_scalar(out=dc[:], in0=Ec[:], scalar1=2.0,
                                    scalar2=2.0, op0=AL.mult, op1=AL.add)
            sc = sb.tile([128, T, 5], F32, tag="sc", bufs=2)
            nc.vector.reciprocal(sc[:].rearrange("p t a -> p (t a)"),
                                 dc[:].rearrange("p t a -> p (t a)"))
            dp = sb.tile([128, T, 5], F32, tag="dp", bufs=2)
            nc.vector.tensor_scalar(out=dp[:], in0=Ep[:], scalar1=2.0,
                                    scalar2=2.0, op0=AL.mult, op1=AL.add)
            sp = sb.tile([128, T, 5], F32, tag="sp", bufs=2)
            nc.vector.reciprocal(sp[:].rearrange("p t a -> p (t a)"),
                                 dp[:].rearrange("p t a -> p (t a)"))
            nc.vector.tensor_tensor(out=outt[:, :, 7:12], in0=sc[:],
                                    in1=sp[:], op=AL.add)
            for t, r0 in enumerate(st["r0"]):
                nc.gpsimd.dma_start(out_d.ap()[r0:r0 + 128, :], outt[:, t, :])

        def emit_LN1C(st):
            return emit_C(emit_LN1(st))

        def emit_LN2E(st):
            return emit_E(emit_LN2(st))

        def emit_attnpool(st):
            return emit_pool(emit_attn(st))

        stages = [emit_A, emit_LN1C, emit_LN2E, emit_attnpool, emit_out]
        nstg = len(stages)
        states = {}
        for tick in range(NSB + nstg - 1):
            for s_idx in reversed(range(nstg)):
                b = tick - s_idx
                if 0 <= b < NSB:
                    if s_idx == 0:
                        states[b] = emit_A(b)
                    else:
                        states[b] = stages[s_idx](states[b])
            states.pop(tick - nstg + 1, None)

    return nc


_CACHE = {}


def _host_prep(inputs):
    f32 = np.float32
    bf = ml_dtypes.bfloat16
    gat_W = inputs["gat_W"].astype(f32)
    gat_a = inputs["gat_a"].astype(f32)
    mha_in_w = inputs["mha_in_w"].astype(f32)
    mha_in_b = inputs["mha_in_b"].astype(f32)
    Wq, Wk, Wv = np.split(mha_in_w, 3, axis=1)
    bq, bk, bv = np.split(mha_in_b, 3)

    def score_mat(query):
        qv = (query.astype(f32) @ Wq + bq).reshape(NH, HD)
        A = np.stack([Wk[:, h * HD:(h + 1) * HD] @ qv[h] for h in range(NH)], 1)
        cK = np.array([bk[h * HD:(h + 1) * HD] @ qv[h] for h in range(NH)], f32)
        return A / np.sqrt(HD), cK / np.sqrt(HD)

    A_emo, ck_emo = score_mat(inputs["emo_query"])
    A_pkl, ck_pkl = score_mat(inputs["pkl_query"])
    gs = gat_W @ np.concatenate(
        [A_emo, A_pkl, gat_a[:H, None], gat_a[H:, None]], 1)
    gv = gat_W @ Wv
    ck = np.concatenate([ck_emo, ck_pkl]).astype(f32)

    ln1_g = inputs["ln1_g"].astype(f32)
    ln1_b = inputs["ln1_b"].astype(f32)
    ln2_g = inputs["ln2_g"].astype(f32)
    ln2_b = inputs["ln2_b"].astype(f32)
    ln1_trivial = np.allclose(ln1_g, 1.0) and np.allclose(ln1_b, 0.0)
    ln2_trivial = np.allclose(ln2_g, 1.0) and np.allclose(ln2_b, 0.0)
    if not (ln1_trivial and ln2_trivial):
        raise NotImplementedError("non-trivial LN affine not supported")

    aw1 = np.stack([np.diag(ln1_g[m]) @ inputs["aW1"][m].astype(f32)
                    for m in range(NMOD)])
    ab1e = inputs["ab1"].astype(f32) + np.einsum(
        "mk,mkn->mn", ln1_b, inputs["aW1"].astype(f32))
    if not np.allclose(ab1e, 0.0):
        raise NotImplementedError("nonzero adapter bias 1 not supported")
    aw2 = inputs["aW2"].astype(f32)
    ab2e = inputs["ab2"].astype(f32)

    mha_out_w = inputs["mha_out_w"].astype(f32)
    mha_out_b = inputs["mha_out_b"].astype(f32)
    rc = np.stack([
        mha_out_b + bv @ mha_out_w + inputs["elp_b"].astype(f32),
        mha_out_b + bv @ mha_out_w + inputs["plp_b"].astype(f32)])

    def norm_rows(g):
        g = g.astype(f32)
        n = np.maximum(np.linalg.norm(g, axis=-1, keepdims=True), 1e-8)
        return g / n

    gn_emo = norm_rows(inputs["guide_emo"])
    gn_pkl = norm_rows(inputs["guide_pkl"])
    pc = np.concatenate([
        inputs["emo_head_w"].astype(f32) * 0.5, gn_emo.T * 0.5,
        inputs["pkl_head_w"].astype(f32), gn_pkl.T], 1)
    pcb = np.concatenate([
        inputs["emo_head_b"].astype(f32) * 0.5, np.zeros(7, f32),
        inputs["pkl_head_b"].astype(f32), np.zeros(5, f32)])

    elp5 = np.tile(inputs["elp_w"].astype(f32) / NMOD, (NMOD, 1))
    plp5 = np.tile(inputs["plp_w"].astype(f32) / NMOD, (NMOD, 1))

    def b16(x):
        return np.ascontiguousarray(np.asarray(x, f32).astype(bf))

    host = dict(
        gv=b16(gv), gs=b16(gs), wo=b16(mha_out_w), pc=b16(pc),
        elp5=b16(elp5), plp5=b16(plp5), aw1=b16(aw1), aw2=b16(aw2),
        bp=b16(inputs["bp"]), ab2e=b16(ab2e), rc=b16(rc),
        pcb=b16(pcb[None, :]), ck=b16(ck[None, :]),
    )
    flags = dict(
        bp=not np.allclose(inputs["bp"].astype(f32), 0.0),
        ab2=not np.allclose(ab2e, 0.0),
        rc=not np.allclose(rc, 0.0),
        pcb=not np.allclose(pcb, 0.0),
        ck=not np.allclose(ck, 0.0),
    )
    return host, flags


def _run(inputs, **spmd_kwargs):
    from concourse.bass_utils import run_bass_kernel_spmd

    host, flags = _host_prep(inputs)
    key = tuple(sorted(flags.items()))
    if key not in _CACHE:
        _CACHE[key] = _build_nc(flags)
    nc = _CACHE[key]

    bf = ml_dtypes.bfloat16
    wp_b16 = {m: np.ascontiguousarray(
        np.asarray(inputs[f"Wp_{m}"], np.float32).astype(bf)) for m in MODS}

    in_maps = []
    for c in range(NCORES):
        r = slice(c * B_CORE, (c + 1) * B_CORE)
        im = {f"feat_{m}": np.ascontiguousarray(
                  inputs[f"feat_{m}"][r], np.float32) for m in MODS}
        for m in MODS:
            im[f"wp_{m}"] = wp_b16[m]
        im["logits"] = np.ascontiguousarray(
            inputs["emo_logits_all"][:, r, :], np.float32)
        im["scores5"] = np.ascontiguousarray(
            inputs["per_scores_all"][:, r, :], np.float32)
        im.update(host)
        in_maps.append(im)

    res = run_bass_kernel_spmd(nc, in_maps, list(range(NCORES)), **spmd_kwargs)
    out = np.concatenate([res.results[c]["out"] for c in range(NCORES)], 0)
    return out, res


def kernel(**inputs):
    return _run(inputs)[0]


# revision 24
# speedup vs baseline: 1.0550x; 1.0550x over previous
"""Trainium2 Bass kernel for nn_MultiModalFusionModelWithAblation.

Strategy: pure data-parallel over 8 NeuronCores (B=16384 -> 2048 rows/core).
Row-major home layout; activations transposed via xbar DMA where a matmul
needs them feature-major (stationary lhsT), bf16 matmul inputs, fp32 PSUM.

v2 structure (vs the v1 baseline):
  - all weights pre-cast to bf16 on HOST (ml_dtypes) and DMAed directly --
    no on-chip cast preamble.
  - super-tiles of SUPER=2 row-tiles: all DVE/Scalar stages run batched
    across the super-tile so activation-table reloads amortize and fixed
    per-instruction overheads shrink.
  - projection relu runs on DVE as tensor_scalar(max) with accum_out
    giving the LN1 row-sum for free (Scalar no longer does the big
    PSUM->SBUF relus).
  - LN2 stats via STT accum_out (sum / sum-of-squares) instead of
    bn_stats; single batched Sqrt per super-tile.
  - sigmoid via Exp + DVE reciprocal (no Sigmoid table).
  - aux logits/scores DMAed straight into transposed layout (strided
    SWDGE) -- no pad/copy/xbar-transpose.
  - pooled attention: per-modality TT chains on contiguous xv slices;
    query 0 on DVE, query 1 on GpSimd (parallel engines).

Host-side algebra (exact, weight-space only) -- unchanged from v1:
  - gat_W folded into the MHA score/value projections (GS / GV).
  - LN affines folded where linear; LN1 rstd dropped entirely (LN2 is
    invariant to per-row positive scales when adapter biases are zero).
  - biases via K=1 ones-outer-product matmuls, skipped when zero.
"""
import sys
import os

sys.path.insert(0, "/opt/trn_rl_repo")

import numpy as np
import orjson
import ml_dtypes
from contextlib import ExitStack

import concourse.bass as bass
import concourse.tile as tile
from concourse import mybir

# ----------------------------------------------------------------------------
# walrus on this toolchain rejects >1 sync-wait per instruction; split excess
# waits onto NoOp carriers on the same engine queue (in-order => equivalent).
_FIXN = [0]


def _fix_bir_waits(d):
    for f in d.get("functions", []):
        for b in f.get("blocks", []):
            insts = b.get("instructions", [])
            if not any(
                len(((i.get("sync_info") or {}).get("on_wait") or [])) > 1
                for i in insts
            ):
                continue
            new = []
            for inst in insts:
                si = inst.get("sync_info")
                waits = (si or {}).get("on_wait") or []
                if len(waits) > 1:
                    for w in waits[:-1]:
                        _FIXN[0] += 1
                        new.append({
                            "engine": inst["engine"], "ins": [], "outs": [],
                            "name": f"wfix-{_FIXN[0]}", "opcode": "NoOp",
                            "debug": inst.get("debug", 0),
                            "sync_info": {"on_update": [], "on_wait": [w]},
                        })
                    si["on_wait"] = [waits[-1]]
                new.append(inst)
            b["instructions"] = new
    return d


if not getattr(bass.Bass, "_waitfix_installed", False):
    _orig_tjb = bass.Bass.to_json_bytes

    def _patched_tjb(self):
        return orjson.dumps(_fix_bir_waits(orjson.loads(_orig_tjb(self))))

    bass.Bass.to_json_bytes = _patched_tjb
    bass.Bass._waitfix_installed = True

# ----------------------------------------------------------------------------
H = 512
NH = 8
HD = 64
NMOD = 5
IN_DIMS = [2048, 1024, 1536, 512, 512]
MODS = ["body", "face", "scene", "audio", "text"]
B_FULL = 16384
NCORES = 8
B_CORE = B_FULL // NCORES          # 2048
NT = B_CORE // 128                 # 16 row tiles per core
SUPER = 2                          # row tiles per super-tile
NSB = NT // SUPER                  # super-tiles per core
ALPHA = 0.2
EPS = 1e-5

F32 = mybir.dt.float32
BF16 = mybir.dt.bfloat16
AF = mybir.ActivationFunctionType
AL = mybir.AluOpType


def _build_nc(flags):
    nc = bass.Bass("TRN2", target_bir_lowering=False, debug=False,
                   num_devices=NCORES)

    # ---- dram io (all weights pre-cast to bf16 on host) ----
    feat_d = [nc.dram_tensor(f"feat_{m}", [B_CORE, ind], F32, kind="ExternalInput")
              for m, ind in zip(MODS, IN_DIMS)]
    wp_d = [nc.dram_tensor(f"wp_{m}", [ind, H], BF16, kind="ExternalInput")
            for m, ind in zip(MODS, IN_DIMS)]
    aw1_d = nc.dram_tensor("aw1", [NMOD, H, H // 2], BF16, kind="ExternalInput")
    aw2_d = nc.dram_tensor("aw2", [NMOD, H // 2, H], BF16, kind="ExternalInput")
    gv_d = nc.dram_tensor("gv", [H, H], BF16, kind="ExternalInput")
    gs_d = nc.dram_tensor("gs", [H, 18], BF16, kind="ExternalInput")
    wo_d = nc.dram_tensor("wo", [H, H], BF16, kind="ExternalInput")
    pc_d = nc.dram_tensor("pc", [H, 24], BF16, kind="ExternalInput")
    elp5_d = nc.dram_tensor("elp5", [35, H], BF16, kind="ExternalInput")
    plp5_d = nc.dram_tensor("plp5", [25, H], BF16, kind="ExternalInput")
    logits_d = nc.dram_tensor("logits", [NMOD, B_CORE, 7], F32, kind="ExternalInput")
    scores5_d = nc.dram_tensor("scores5", [NMOD, B_CORE, 5], F32, kind="ExternalInput")
    # optional bias rows (always declared; tiny)
    bp_d = nc.dram_tensor("bp", [NMOD, H], BF16, kind="ExternalInput")
    ab2_d = nc.dram_tensor("ab2e", [NMOD, H], BF16, kind="ExternalInput")
    rc_d = nc.dram_tensor("rc", [2, H], BF16, kind="ExternalInput")
    pcb_d = nc.dram_tensor("pcb", [1, 24], BF16, kind="ExternalInput")
    ck_d = nc.dram_tensor("ck", [1, 16], BF16, kind="ExternalInput")
    out_d = nc.dram_tensor("out", [B_CORE, 12], F32, kind="ExternalOutput")

    NK = [ind // 128 for ind in IN_DIMS]
    T = SUPER

    with tile.TileContext(nc) as tc, ExitStack() as ctx:
        wpool = ctx.enter_context(tc.tile_pool(name="weights", bufs=1))
        sb = ctx.enter_context(tc.tile_pool(name="work", bufs=1))
        ps = ctx.enter_context(tc.tile_pool(name="psum", bufs=1, space="PSUM"))

        # ---- one-time weight loads: direct bf16 HWDGE on the scalar queue
        def _wload(dram_ap, shape, tag):
            t = wpool.tile(shape, BF16, tag=tag)
            nc.scalar.dma_start(t[:], dram_ap)
            return t

        wp_bf = [
            _wload(wp_d[m].ap().rearrange("(k c) n -> c k n", c=128),
                   [128, NK[m], H], f"wp{m}")
            for m in range(NMOD)
        ]
        aw1_bf = _wload(aw1_d.ap().rearrange("m (k c) n -> c (m k) n", c=128),
                        [128, NMOD * 4, H // 2], "aw1")
        aw1_bf = aw1_bf[:].rearrange("c (m k) n -> c m k n", m=NMOD)
        aw2_bf = _wload(aw2_d.ap().rearrange("m (k c) n -> c (m k) n", c=128),
                        [128, NMOD * 2, H], "aw2")
        aw2_bf = aw2_bf[:].rearrange("c (m k) n -> c m k n", m=NMOD)
        gv_bf = _wload(gv_d.ap().rearrange("(k c) n -> c k n", c=128),
                       [128, 4, H], "gv")
        gs_bf = _wload(gs_d.ap().rearrange("(k c) n -> c k n", c=128),
                       [128, 4, 18], "gs")
        wo_bf = _wload(wo_d.ap().rearrange("(k c) n -> c k n", c=128),
                       [128, 4, H], "wo")
        pc_bf = _wload(pc_d.ap().rearrange("(k c) n -> c k n", c=128),
                       [128, 4, 24], "pc")
        elp5_bf = _wload(elp5_d.ap(), [35, H], "elp5")
        plp5_bf = _wload(plp5_d.ap(), [25, H], "plp5")

        eps_t = wpool.tile([128, 1], F32, tag="eps")
        nc.vector.memset(eps_t[:], EPS)

        ones1 = None
        if any([flags["bp"], flags["ab2"], flags["rc"], flags["pcb"],
                flags["ck"]]):
            ones1 = wpool.tile([1, 128], BF16, tag="ones1")
            nc.vector.memset(ones1[:], 1.0)

        def _bias_row(dram_ap, n, tag):
            t = wpool.tile([1, n], BF16, tag=tag)
            nc.gpsimd.dma_start(t[:], dram_ap)
            return t

        bp_bf = _bias_row(bp_d.ap().rearrange("m n -> 1 (m n)"), NMOD * H, "bp") \
            if flags["bp"] else None
        ab2_bf = _bias_row(ab2_d.ap().rearrange("m n -> 1 (m n)"), NMOD * H, "ab2") \
            if flags["ab2"] else None
        rc_bf = _bias_row(rc_d.ap().rearrange("q n -> 1 (q n)"), 2 * H, "rc") \
            if flags["rc"] else None
        pcb_bf = _bias_row(pcb_d.ap()[:], 24, "pcb") if flags["pcb"] else None
        ck_t = None
        if flags["ck"]:
            ck_row = _bias_row(ck_d.ap()[:], 16, "ckrow")
            ck_ps = ps.tile([128, H], F32, tag="psB")
            nc.tensor.matmul(ck_ps[:, 0:16], lhsT=ones1[:], rhs=ck_row[:],
                             start=True, stop=True)
            ck_t = wpool.tile([128, 16], F32, tag="ckt")
            nc.vector.tensor_copy(out=ck_t[:], in_=ck_ps[:, 0:16])

        # persistent zero-padded staging for the aux-logit transposes
        auxpad = []
        for i in range(2):
            t = wpool.tile([128, T, 2, 128], BF16, tag=f"auxpad{i}")
            nc.vector.memset(t[:], 0.0)
            auxpad.append(t)

        # xbar queue alternation for DMA transposes
        _tq = [0]

        def _tqueue():
            _tq[0] ^= 1
            return nc.sync

        # ---------------- pipeline stages (per super-tile of T row-tiles) ---
        def emit_A(sb_i):
            """aux + feat loads, transposes, projection, relu+rowsum."""
            st = {"sb": sb_i}
            tiles = [sb_i * T + t for t in range(T)]
            st["r0"] = [ti * 128 for ti in tiles]

            h4 = sb.tile([128, T, NMOD, H], BF16, tag="h4", bufs=1)
            hsum = sb.tile([128, T * NMOD], F32, tag="hsum", bufs=2)
            for t, r0 in enumerate(st["r0"]):
                for grp in ([0], [2], [1, 3, 4]):
                    gw = sum(IN_DIMS[m] for m in grp)
                    fz = sb.tile([128, IN_DIMS[0]], BF16, tag="fz", bufs=1)
                    off = 0
                    for m in grp:
                        nc.gpsimd.dma_start(
                            fz[:, off:off + IN_DIMS[m]],
                            feat_d[m].ap()[r0:r0 + 128, :])
                        off += IN_DIMS[m]
                    fT = sb.tile([128, NK[0], 128], BF16, tag="fT", bufs=2)
                    _tqueue().dma_start(fT[:, :gw // 128, :], fz[:, :gw],
                                        transpose=True)
                    koff = 0
                    for m in grp:
                        nk = NK[m]
                        h_ps = ps.tile([128, H], F32, tag="psA", bufs=2)
                        if flags["bp"]:
                            nc.tensor.matmul(h_ps[:], lhsT=ones1[:],
                                             rhs=bp_bf[:, m * H:(m + 1) * H],
                                             start=True, stop=False)
                        for k in range(nk):
                            nc.tensor.matmul(h_ps[:], lhsT=fT[:, koff + k, :],
                                             rhs=wp_bf[m][:, k, :],
                                             start=(k == 0 and not flags["bp"]),
                                             stop=(k == nk - 1))
                        koff += nk
                        # relu + row-sum in one DVE op
                        idx = t * NMOD + m
                        nc.vector.tensor_scalar(
                            out=h4[:, t, m, :], in0=h_ps[:], scalar1=0.0,
                            scalar2=0.0, op0=AL.max, op1=AL.add,
                            accum_out=hsum[:, idx:idx + 1])
            negmu = sb.tile([128, T * NMOD], F32, tag="negmu", bufs=2)
            nc.vector.tensor_scalar_mul(negmu[:], hsum[:], -1.0 / H)
            st["h4"] = h4
            st["negmu"] = negmu
            return st

        def emit_LN1(st):
            h4, negmu = st["h4"], st["negmu"]
            hln = sb.tile([128, T, NMOD, H], BF16, tag="hln", bufs=1)
            for t in range(T):
                for m in range(NMOD):
                    idx = t * NMOD + m
                    nc.vector.tensor_scalar(
                        out=hln[:, t, m, :], in0=h4[:, t, m, :],
                        scalar1=negmu[:, idx:idx + 1], scalar2=None,
                        op0=AL.add)
            hT = sb.tile([128, T, NMOD * 4, 128], BF16, tag="hT", bufs=1)
            for t in range(T):
                _tqueue().dma_start(
                    hT[:, t], hln[:, t].rearrange("p m h -> p (m h)"),
                    transpose=True)
            st["hln"] = hln
            st["hT"] = hT
            return st

        def emit_C(st):
            hln, hT = st["hln"], st["hT"]
            assert not flags.get("ab1", False)
            # adapter hidden, feature-major, batched over the super-tile rows
            zT = sb.tile([128, NMOD * 2, T * 128], BF16, tag="zT", bufs=1)
            for m in range(NMOD):
                for c in range(2):
                    z_ps = ps.tile([128, H], F32, tag="psB", bufs=2)
                    for k in range(4):
                        nc.tensor.matmul(
                            z_ps[:, :T * 128],
                            lhsT=aw1_bf[:, m, k, c * 128:(c + 1) * 128],
                            rhs=hT[:, :, m * 4 + k, :],
                            start=(k == 0), stop=(k == 3))
                    nc.scalar.activation(zT[:, m * 2 + c, :],
                                         z_ps[:, :T * 128], AF.Relu)
            # adapter out + residual; LN2 stats via accum_out
            u4 = sb.tile([128, T, NMOD, H], BF16, tag="u4", bufs=1)
            su = sb.tile([128, T * NMOD], F32, tag="su", bufs=2)
            su2 = sb.tile([128, T * NMOD], F32, tag="su2", bufs=2)
            for t in range(T):
                for m in range(NMOD):
                    a2_ps = ps.tile([128, H], F32, tag="psC", bufs=2)
                    if flags["ab2"]:
                        nc.tensor.matmul(a2_ps[:], lhsT=ones1[:],
                                         rhs=ab2_bf[:, m * H:(m + 1) * H],
                                         start=True, stop=False)
                    for k in range(2):
                        nc.tensor.matmul(
                            a2_ps[:],
                            lhsT=zT[:, m * 2 + k, t * 128:(t + 1) * 128],
                            rhs=aw2_bf[:, m, k, :],
                            start=(k == 0 and not flags["ab2"]),
                            stop=(k == 1))
                    idx = t * NMOD + m
                    nc.vector.scalar_tensor_tensor(
                        out=u4[:, t, m, :], in0=a2_ps[:], scalar=1.0,
                        in1=hln[:, t, m, :], op0=AL.mult, op1=AL.add,
                        accum_out=su[:, idx:idx + 1])
                    s2 = sb.tile([128, H], BF16, tag="scr", bufs=1)
                    nc.vector.scalar_tensor_tensor(
                        out=s2[:], in0=u4[:, t, m, :], scalar=1.0,
                        in1=u4[:, t, m, :], op0=AL.mult, op1=AL.mult,
                        accum_out=su2[:, idx:idx + 1])
            # mu, var, rstd (batched tiny ops + one Sqrt)
            mu = sb.tile([128, T * NMOD], F32, tag="mu", bufs=2)
            nc.vector.tensor_scalar_mul(mu[:], su[:], 1.0 / H)
            m2 = sb.tile([128, T * NMOD], F32, tag="m2", bufs=2)
            nc.vector.tensor_tensor(out=m2[:], in0=mu[:], in1=mu[:],
                                    op=AL.mult)
            var = sb.tile([128, T * NMOD], F32, tag="var", bufs=2)
            nc.vector.scalar_tensor_tensor(
                out=var[:], in0=su2[:], scalar=1.0 / H, in1=m2[:],
                op0=AL.mult, op1=AL.subtract)
            sd = sb.tile([128, T * NMOD], F32, tag="sd", bufs=2)
            nc.scalar.activation(sd[:], var[:], AF.Sqrt, bias=eps_t[:])
            rstd = sb.tile([128, T * NMOD], F32, tag="rstd", bufs=2)
            nc.vector.reciprocal(rstd[:], sd[:])
            st["u4"] = u4
            st["mu"] = mu
            st["rstd"] = rstd
            return st

        def emit_LN2(st):
            u4, mu, rstd = st["u4"], st["mu"], st["rstd"]
            xT = sb.tile([128, T, NMOD * 4, 128], BF16, tag="xT", bufs=1)
            for t in range(T):
                xm = sb.tile([128, NMOD, H], BF16, tag="xm", bufs=1)
                for m in range(NMOD):
                    idx = t * NMOD + m
                    nc.vector.tensor_scalar(
                        out=xm[:, m, :], in0=u4[:, t, m, :],
                        scalar1=mu[:, idx:idx + 1],
                        scalar2=rstd[:, idx:idx + 1],
                        op0=AL.subtract, op1=AL.mult)
                _tqueue().dma_start(
                    xT[:, t], xm[:].rearrange("p m h -> p (m h)"),
                    transpose=True)
            st["xT"] = xT
            return st

        def emit_E(st):
            xT = st["xT"]
            xv4 = sb.tile([128, T, NMOD, H], BF16, tag="xv4", bufs=2)
            xss = sb.tile([128, T, NMOD, 18], F32, tag="xss", bufs=2)
            for t in range(T):
                for m in range(NMOD):
                    xv_ps = ps.tile([128, H], F32, tag="psD", bufs=2)
                    xs_ps = ps.tile([128, H], F32, tag="psB", bufs=2)
                    for k in range(4):
                        nc.tensor.matmul(xv_ps[:], lhsT=xT[:, t, m * 4 + k, :],
                                         rhs=gv_bf[:, k, :],
                                         start=(k == 0), stop=(k == 3))
                        nc.tensor.matmul(xs_ps[:, 0:18],
                                         lhsT=xT[:, t, m * 4 + k, :],
                                         rhs=gs_bf[:, k, :],
                                         start=(k == 0), stop=(k == 3))
                    nc.scalar.activation(xv4[:, t, m, :], xv_ps[:], AF.Copy)
                    nc.vector.tensor_copy(out=xss[:, t, m, :],
                                          in_=xs_ps[:, 0:18])
            st["xv4"] = xv4
            st["xss"] = xss
            return st

        def emit_attn(st):
            xss = st["xss"]
            s1 = xss[:, :, :, 16]                       # [128,T,5]
            s2 = xss[:, :, :, 17]
            e4 = sb.tile([128, T, 5, 5], F32, tag="e4", bufs=1)
            nc.vector.tensor_tensor(
                out=e4[:],
                in0=s2[:, :, None, :].broadcast_to([128, T, 5, 5]),
                in1=s1[:, :, :, None].broadcast_to([128, T, 5, 5]),
                op=AL.add)
            el = sb.tile([128, T, 25], F32, tag="el", bufs=1)
            nc.vector.scalar_tensor_tensor(
                out=el[:], in0=e4[:].rearrange("p t a b -> p t (a b)"),
                scalar=ALPHA,
                in1=e4[:].rearrange("p t a b -> p t (a b)"),
                op0=AL.mult, op1=AL.max)
            ex = sb.tile([128, T, 5, 5], F32, tag="ex", bufs=1)
            nc.scalar.activation(ex[:].rearrange("p t a b -> p t (a b)"),
                                 el[:], AF.Exp)
            den = sb.tile([128, T, 5], F32, tag="den", bufs=1)
            nc.vector.tensor_reduce(out=den[:], in_=ex[:],
                                    axis=mybir.AxisListType.X, op=AL.add)
            rden = sb.tile([128, T, 5], F32, tag="rden", bufs=1)
            nc.vector.reciprocal(rden[:], den[:])
            attn = sb.tile([128, T, 5, 5], F32, tag="attn", bufs=1)
            nc.vector.tensor_tensor(
                out=attn[:], in0=ex[:],
                in1=rden[:, :, :, None].broadcast_to([128, T, 5, 5]),
                op=AL.mult)

            tmp400 = sb.tile([128, T, 16, 5, 5], BF16, tag="tmp400", bufs=1)
            S4 = sb.tile([128, T, 16, 5], F32, tag="S4", bufs=1)
            for t in range(T):
                nc.vector.tensor_tensor(
                    out=tmp400[:, t],
                    in0=xss[:, t, :, 0:16].rearrange("p j q -> p q j")
                        [:, :, None, :].broadcast_to([128, 16, 5, 5]),
                    in1=attn[:, t][:, None, :, :].broadcast_to([128, 16, 5, 5]),
                    op=AL.mult)
                nc.vector.tensor_reduce(out=S4[:, t], in_=tmp400[:, t],
                                        axis=mybir.AxisListType.X, op=AL.add)
            if flags["ck"]:
                nc.vector.tensor_tensor(
                    out=S4[:], in0=S4[:],
                    in1=ck_t[:][:, None, :, None]
                        .broadcast_to([128, T, 16, 5]), op=AL.add)
            ES = sb.tile([128, T, 16, 5], F32, tag="ES", bufs=1)
            nc.scalar.activation(ES[:].rearrange("p t a b -> p t (a b)"),
                                 S4[:].rearrange("p t a b -> p t (a b)"),
                                 AF.Exp)
            den16 = sb.tile([128, T, 16], F32, tag="den16", bufs=1)
            nc.vector.tensor_reduce(out=den16[:], in_=ES[:],
                                    axis=mybir.AxisListType.X, op=AL.add)
            rden16 = sb.tile([128, T, 16], F32, tag="rden16", bufs=1)
            nc.vector.reciprocal(rden16[:], den16[:])
            P4 = sb.tile([128, T, 16, 5], BF16, tag="P4", bufs=1)
            nc.vector.tensor_tensor(
                out=P4[:], in0=ES[:],
                in1=rden16[:, :, :, None].broadcast_to([128, T, 16, 5]),
                op=AL.mult)
            tmp2 = sb.tile([128, T, 16, 5, 5], BF16, tag="tmp400", bufs=1)
            W4 = sb.tile([128, T, 16, 5], BF16, tag="W4", bufs=2)
            for t in range(T):
                nc.vector.tensor_tensor(
                    out=tmp2[:, t],
                    in0=P4[:, t][:, :, None, :].broadcast_to([128, 16, 5, 5]),
                    in1=attn[:, t].rearrange("p n j -> p j n")
                        [:, None, :, :].broadcast_to([128, 16, 5, 5]),
                    op=AL.mult)
                with nc.allow_low_precision("5-term pooled-attn sums"):
                    nc.vector.tensor_reduce(out=W4[:, t], in_=tmp2[:, t],
                                            axis=mybir.AxisListType.X, op=AL.add)
            st["W4"] = W4
            return st

        def _pool_q(eng, xv4, W4, o4, q, tags):
            """o4[:, :, q, :] = sum_j W4[:, :, q-heads, j] * xv4[:, :, j, :]"""
            def wv(j):
                return W4[:, :, q * 8:(q + 1) * 8, j:j + 1] \
                    .broadcast_to([128, T, 8, HD])

            def xv(j):
                return xv4[:, :, j, :].rearrange("p t (h d) -> p t h d", h=8)

            pa = sb.tile([128, T, 8, HD], BF16, tag=tags[0], bufs=1)
            pb = sb.tile([128, T, 8, HD], BF16, tag=tags[1], bufs=1)
            with nc.allow_low_precision("5-term pooled-attn sums"):
                eng.tensor_tensor(out=pa[:], in0=xv(0), in1=wv(0), op=AL.mult)
                eng.tensor_tensor(out=pb[:], in0=xv(1), in1=wv(1), op=AL.mult)
                eng.tensor_tensor(out=pa[:], in0=pa[:], in1=pb[:], op=AL.add)
                pb2 = sb.tile([128, T, 8, HD], BF16, tag=tags[1], bufs=1)
                eng.tensor_tensor(out=pb2[:], in0=xv(2), in1=wv(2), op=AL.mult)
                eng.tensor_tensor(out=pa[:], in0=pa[:], in1=pb2[:], op=AL.add)
                pb3 = sb.tile([128, T, 8, HD], BF16, tag=tags[1], bufs=1)
                eng.tensor_tensor(out=pb3[:], in0=xv(3), in1=wv(3), op=AL.mult)
                eng.tensor_tensor(out=pa[:], in0=pa[:], in1=pb3[:], op=AL.add)
                pb4 = sb.tile([128, T, 8, HD], BF16, tag=tags[1], bufs=1)
                eng.tensor_tensor(out=pb4[:], in0=xv(4), in1=wv(4), op=AL.mult)
                eng.tensor_tensor(
                    out=o4[:, :, q, :].rearrange("p t (h d) -> p t h d", h=8),
                    in0=pa[:], in1=pb4[:], op=AL.add)

        def emit_pool(st):
            xv4, W4 = st["xv4"], st["W4"]
            # aux logits/scores: contiguous row-major loads into the
            # zero-padded staging, one xbar transpose per super-tile
            pad = auxpad[st["sb"] % 2]
            for t, r0 in enumerate(st["r0"]):
                lg = sb.tile([128, NMOD, 7], F32, tag="lg", bufs=2)
                nc.gpsimd.dma_start(
                    lg[:], logits_d.ap()[:, r0:r0 + 128, :]
                    .rearrange("m r c -> r m c"))
                nc.vector.tensor_copy(out=pad[:, t, 0, 0:35],
                                      in_=lg[:].rearrange("p m c -> p (m c)"))
                sc = sb.tile([128, NMOD, 5], F32, tag="sc", bufs=2)
                nc.gpsimd.dma_start(
                    sc[:], scores5_d.ap()[:, r0:r0 + 128, :]
                    .rearrange("m r c -> r m c"))
                nc.vector.tensor_copy(out=pad[:, t, 1, 0:25],
                                      in_=sc[:].rearrange("p m c -> p (m c)"))
            auxT = sb.tile([128, T, 2, 128], BF16, tag="auxT", bufs=3)
            nc.sync.dma_start(auxT[:].rearrange("p t a b -> p (t a) b"),
                              pad[:].rearrange("p t a b -> p (t a b)"),
                              transpose=True)
            st["auxT"] = auxT
            o4 = sb.tile([128, T, 2, H], BF16, tag="o4", bufs=1)
            _pool_q(nc.vector, xv4, W4, o4, 0, ("vpa", "vpb"))
            _pool_q(nc.vector, xv4, W4, o4, 1, ("vpa", "vpb"))
            st["o4"] = o4
            return st

        def emit_out(st):
            o4, auxT = st["o4"], st["auxT"]
            oT = sb.tile([128, T, 8, 128], BF16, tag="oT", bufs=1)
            for t in range(T):
                _tqueue().dma_start(
                    oT[:, t], o4[:, t].rearrange("p a b -> p (a b)"),
                    transpose=True)
            rep4 = sb.tile([128, T, 2, H], BF16, tag="rep4", bufs=1)
            n2 = sb.tile([128, T * 2], F32, tag="n2", bufs=2)
            for t in range(T):
                for q in range(2):
                    repr_ps = ps.tile([128, H], F32, tag="psD", bufs=2)
                    if flags["rc"]:
                        nc.tensor.matmul(repr_ps[:], lhsT=ones1[:],
                                         rhs=rc_bf[:, q * H:(q + 1) * H],
                                         start=True, stop=False)
                    for k in range(4):
                        nc.tensor.matmul(repr_ps[:], lhsT=oT[:, t, q * 4 + k, :],
                                         rhs=wo_bf[:, k, :],
                                         start=(k == 0 and not flags["rc"]),
                                         stop=False)
                    if q == 0:
                        nc.tensor.matmul(repr_ps[:],
                                         lhsT=auxT[0:35, t, 0, :],
                                         rhs=elp5_bf[:], start=False, stop=True)
                    else:
                        nc.tensor.matmul(repr_ps[:],
                                         lhsT=auxT[0:25, t, 1, :],
                                         rhs=plp5_bf[:], start=False, stop=True)
                    nc.scalar.activation(rep4[:, t, q, :], repr_ps[:], AF.Copy)
                    sq = sb.tile([128, H], BF16, tag="scr2", bufs=1)
                    idx = t * 2 + q
                    nc.vector.scalar_tensor_tensor(
                        out=sq[:], in0=rep4[:, t, q, :], scalar=1.0,
                        in1=rep4[:, t, q, :], op0=AL.mult, op1=AL.mult,
                        accum_out=n2[:, idx:idx + 1])
            nrm = sb.tile([128, T * 2], F32, tag="nrm", bufs=2)
            nc.scalar.activation(nrm[:], n2[:], AF.Sqrt)
            nc.vector.tensor_scalar_max(nrm[:], nrm[:], 1e-8)
            rn = sb.tile([128, T * 2], F32, tag="rn", bufs=2)
            nc.vector.reciprocal(rn[:], nrm[:])
            negrn = sb.tile([128, T * 2], F32, tag="negrn", bufs=2)
            nc.vector.tensor_scalar_mul(negrn[:], rn[:], -1.0)

            rT = sb.tile([128, T, 8, 128], BF16, tag="rT", bufs=1)
            for t in range(T):
                _tqueue().dma_start(
                    rT[:, t], rep4[:, t].rearrange("p a b -> p (a b)"),
                    transpose=True)
            pred4 = sb.tile([128, T, 24], F32, tag="pred4", bufs=2)
            for t in range(T):
                pred_ps = ps.tile([128, H], F32, tag="psB", bufs=2)
                if flags["pcb"]:
                    nc.tensor.matmul(pred_ps[:, 0:24], lhsT=ones1[:],
                                     rhs=pcb_bf[:], start=True, stop=False)
                for q in range(2):
                    cols = slice(0, 14) if q == 0 else slice(14, 24)
                    for k in range(4):
                        nc.tensor.matmul(pred_ps[:, cols],
                                         lhsT=rT[:, t, q * 4 + k, :],
                                         rhs=pc_bf[:, k, cols],
                                         start=(k == 0 and not flags["pcb"]),
                                         stop=(k == 3))
                nc.vector.tensor_copy(out=pred4[:, t, :],
                                      in_=pred_ps[:, 0:24])

            outt = sb.tile([128, T, 12], F32, tag="outt", bufs=2)
            # emo half: pred[0:7]*0.5-folded + cos*0.5-folded
            c7 = sb.tile([128, T, 7], F32, tag="c7", bufs=2)
            nc.vector.tensor_tensor(
                out=c7[:], in0=pred4[:, :, 7:14],
                in1=rn[:].rearrange("p (t q) -> p t q", t=T)[:, :, 0:1]
                    .broadcast_to([128, T, 7]),
                op=AL.mult)
            nc.vector.tensor_tensor(out=outt[:, :, 0:7], in0=c7[:],
                                    in1=pred4[:, :, 0:7], op=AL.add)
            # pkl half: (sigmoid(pred14:19) + sigmoid(cos))*0.5 via Exp
            Ec = sb.tile([128, T, 5], F32, tag="Ec", bufs=2)
            for t in range(T):
                idx = t * 2 + 1
                nc.scalar.activation(Ec[:, t, :], pred4[:, t, 19:24], AF.Exp,
                                     scale=negrn[:, idx:idx + 1])
            Ep = sb.tile([128, T, 5], F32, tag="Ep", bufs=2)
            nc.scalar.activation(Ep[:], pred4[:, :, 14:19], AF.Exp, scale=-1.0)
            dc = sb.tile([128, T, 5], F32, tag="dc", bufs=2)
            nc.vector.tensor_scalar(out=dc[:], in0=Ec[:], scalar1=2.0,
                                    scalar2=2.0, op0=AL.mult, op1=AL.add)
            sc = sb.tile([128, T, 5], F32, tag="sc", bufs=2)
            nc.vector.reciprocal(sc[:].rearrange("p t a -> p (t a)"),
                                 dc[:].rearrange("p t a -> p (t a)"))
            dp = sb.tile([128, T, 5], F32, tag="dp", bufs=2)
            nc.vector.tensor_scalar(out=dp[:], in0=Ep[:], scalar1=2.0,
                                    scalar2=2.0, op0=AL.mult, op1=AL.add)
            sp = sb.tile([128, T, 5], F32, tag="sp", bufs=2)
            nc.vector.reciprocal(sp[:].rearrange("p t a -> p (t a)"),
                                 dp[:].rearrange("p t a -> p (t a)"))
            nc.vector.tensor_tensor(out=outt[:, :, 7:12], in0=sc[:],
                                    in1=sp[:], op=AL.add)
            for t, r0 in enumerate(st["r0"]):
                nc.gpsimd.dma_start(out_d.ap()[r0:r0 + 128, :], outt[:, t, :])

        stages = [emit_A, emit_LN1, emit_C, emit_LN2, emit_E, emit_attn,
                  emit_pool, emit_out]
        nstg = len(stages)
        states = {}
        for tick in range(NSB + nstg - 1):
            for s_idx in reversed(range(nstg)):
                b = tick - s_idx
                if 0 <= b < NSB:
                    if s_idx == 0:
                        states[b] = emit_A(b)
                    else:
                        states[b] = stages[s_idx](states[b])
            states.pop(tick - nstg + 1, None)

    return nc


_CACHE = {}


def _host_prep(inputs):
    f32 = np.float32
    bf = ml_dtypes.bfloat16
    gat_W = inputs["gat_W"].astype(f32)
    gat_a = inputs["gat_a"].astype(f32)
    mha_in_w = inputs["mha_in_w"].astype(f32)
    mha_in_b = inputs["mha_in_b"].astype(f32)
    Wq, Wk, Wv = np.split(mha_in_w, 3, axis=1)
    bq, bk, bv = np.split(mha_in_b, 3)

    def score_mat(query):
        qv = (query.astype(f32) @ Wq + bq).reshape(NH, HD)
        A = np.stack([Wk[:, h * HD:(h + 1) * HD] @ qv[h] for h in range(NH)], 1)
        cK = np.array([bk[h * HD:(h + 1) * HD] @ qv[h] for h in range(NH)], f32)
        return A / np.sqrt(HD), cK / np.sqrt(HD)

    A_emo, ck_emo = score_mat(inputs["emo_query"])
    A_pkl, ck_pkl = score_mat(inputs["pkl_query"])
    gs = gat_W @ np.concatenate(
        [A_emo, A_pkl, gat_a[:H, None], gat_a[H:, None]], 1)
    gv = gat_W @ Wv
    ck = np.concatenate([ck_emo, ck_pkl]).astype(f32)

    ln1_g = inputs["ln1_g"].astype(f32)
    ln1_b = inputs["ln1_b"].astype(f32)
    ln2_g = inputs["ln2_g"].astype(f32)
    ln2_b = inputs["ln2_b"].astype(f32)
    ln1_trivial = np.allclose(ln1_g, 1.0) and np.allclose(ln1_b, 0.0)
    ln2_trivial = np.allclose(ln2_g, 1.0) and np.allclose(ln2_b, 0.0)
    if not (ln1_trivial and ln2_trivial):
        raise NotImplementedError("non-trivial LN affine not supported")

    aw1 = np.stack([np.diag(ln1_g[m]) @ inputs["aW1"][m].astype(f32)
                    for m in range(NMOD)])
    ab1e = inputs["ab1"].astype(f32) + np.einsum(
        "mk,mkn->mn", ln1_b, inputs["aW1"].astype(f32))
    if not np.allclose(ab1e, 0.0):
        raise NotImplementedError("nonzero adapter bias 1 not supported")
    aw2 = inputs["aW2"].astype(f32)
    ab2e = inputs["ab2"].astype(f32)

    mha_out_w = inputs["mha_out_w"].astype(f32)
    mha_out_b = inputs["mha_out_b"].astype(f32)
    rc = np.stack([
        mha_out_b + bv @ mha_out_w + inputs["elp_b"].astype(f32),
        mha_out_b + bv @ mha_out_w + inputs["plp_b"].astype(f32)])

    def norm_rows(g):
        g = g.astype(f32)
        n = np.maximum(np.linalg.norm(g, axis=-1, keepdims=True), 1e-8)
        return g / n

    gn_emo = norm_rows(inputs["guide_emo"])
    gn_pkl = norm_rows(inputs["guide_pkl"])
    pc = np.concatenate([
        inputs["emo_head_w"].astype(f32) * 0.5, gn_emo.T * 0.5,
        inputs["pkl_head_w"].astype(f32), gn_pkl.T], 1)
    pcb = np.concatenate([
        inputs["emo_head_b"].astype(f32) * 0.5, np.zeros(7, f32),
        inputs["pkl_head_b"].astype(f32), np.zeros(5, f32)])

    elp5 = np.tile(inputs["elp_w"].astype(f32) / NMOD, (NMOD, 1))
    plp5 = np.tile(inputs["plp_w"].astype(f32) / NMOD, (NMOD, 1))

    def b16(x):
        return np.ascontiguousarray(np.asarray(x, f32).astype(bf))

    host = dict(
        gv=b16(gv), gs=b16(gs), wo=b16(mha_out_w), pc=b16(pc),
        elp5=b16(elp5), plp5=b16(plp5), aw1=b16(aw1), aw2=b16(aw2),
        bp=b16(inputs["bp"]), ab2e=b16(ab2e), rc=b16(rc),
        pcb=b16(pcb[None, :]), ck=b16(ck[None, :]),
    )
    flags = dict(
        bp=not np.allclose(inputs["bp"].astype(f32), 0.0),
        ab2=not np.allclose(ab2e, 0.0),
        rc=not np.allclose(rc, 0.0),
        pcb=not np.allclose(pcb, 0.0),
        ck=not np.allclose(ck, 0.0),
    )
    return host, flags


def _run(inputs, **spmd_kwargs):
    from concourse.bass_utils import run_bass_kernel_spmd

    host, flags = _host_prep(inputs)
    key = tuple(sorted(flags.items()))
    if key not in _CACHE:
        _CACHE[key] = _build_nc(flags)
    nc = _CACHE[key]

    bf = ml_dtypes.bfloat16
    wp_b16 = {m: np.ascontiguousarray(
        np.asarray(inputs[f"Wp_{m}"], np.float32).astype(bf)) for m in MODS}

    in_maps = []
    for c in range(NCORES):
        r = slice(c * B_CORE, (c + 1) * B_CORE)
        im = {f"feat_{m}": np.ascontiguousarray(
                  inputs[f"feat_{m}"][r], np.float32) for m in MODS}
        for m in MODS:
            im[f"wp_{m}"] = wp_b16[m]
        im["logits"] = np.ascontiguousarray(
            inputs["emo_logits_all"][:, r, :], np.float32)
        im["scores5"] = np.ascontiguousarray(
            inputs["per_scores_all"][:, r, :], np.float32)
        im.update(host)
        in_maps.append(im)

    res = run_bass_kernel_spmd(nc, in_maps, list(range(NCORES)), **spmd_kwargs)
    out = np.concatenate([res.results[c]["out"] for c in range(NCORES)], 0)
    return out, res


def kernel(**inputs):
    return _run(inputs)[0]
